# revision 2
# baseline (speedup 1.0000x reference)
"""Round 4: host-padded x (W+2), strided matmul views, no on-chip copies.

Trainium2 Bass kernel for a 3x3 stride-1 pad-1 conv:
x (32,128,64,64) f32, weight (256,128,3,3) f32, bias (256,) f32
-> out (32,256,64,64) f32.

Data-parallel over batch across 8 NeuronCores (4 samples each); conv as
9 shifted fp16 matmuls per 8-row output tile accumulating in fp32 PSUM.

Horizontal padding comes for free: the host pads x to width 66 with zero
columns, so tap (kh, kw) is just the strided SBUF view
st[:, r0:r0+8, kw:kw+64] (row stride 66) - no shifted copies, no memsets.
Vertical padding via range-restricted matmuls on edge tiles (taps ordered
so the first matmul of each accumulation group covers the full PSUM range).

Weights are repacked on the host to wt[ci, tap, co_half] fp16 with the
first-needed taps (cb0 kh1) first; the first weight chunk and the first
x rows go out first on the sync DGE ring so conv matmuls can start as
soon as the ring delivers (~2.3us after issue). A few identity transposes
keep the PE busy from the ordering barrier until then, which also starts
the HAM clock ramp early.

Output pairs (two 8-row tiles) are bias-added on the scalar engine and
stored fp16, alternating between both DGE rings; the final tile is split
4+2+2 rows so the closing matmul->drain->store->completion chain is short.
"""

import numpy as np

import concourse.bass as bass
from concourse import bacc
import concourse.mybir as mybir
import concourse.tile as tile
from concourse.bass_utils import run_bass_kernel_spmd
from concourse.masks import make_identity

N_CORES = 8
B_FULL = 32
B_LOCAL = B_FULL // N_CORES  # 4
CI = 128
CO = 256
H = W = 64
WP = W + 2  # host-padded width (zero col at 0 and 65)
ROWS = 8  # output rows per PSUM tile -> free dim 8*64 = 512
N_T = H // ROWS  # 8
F32 = mybir.dt.float32
F16 = mybir.dt.float16

# wt tap layout: [cb0 kh1 | cb0 kh2 | cb0 kh0 | cb1 kh0 | cb1 kh1 | cb1 kh2]
# (first-needed taps first; each group is kw=0,1,2)
TAP = {(0, 1): 0, (0, 2): 3, (0, 0): 6, (1, 0): 9, (1, 1): 12, (1, 2): 15}

# Sample-0 x arrives in four row chunks so compute can start early.
# Tile t needs input rows 8t-1 .. 8t+8.
S0_CHUNKS = [(0, 9), (9, 8), (17, 16), (33, 31)]


def build_nc():
    nc = bacc.Bacc()
    x_d = nc.dram_tensor("x", [B_LOCAL, CI, H, WP], F16, kind="ExternalInput")
    w_d = nc.dram_tensor("wt", [CI, 18, CO // 2], F16, kind="ExternalInput")
    b_d = nc.dram_tensor("bias", [CO], F32, kind="ExternalInput")
    o_d = nc.dram_tensor("out", [B_LOCAL, CO, H, W], F16, kind="ExternalOutput")

    with tile.TileContext(nc) as tc:
        with (
            tc.tile_pool(name="const", bufs=1) as const,
            tc.tile_pool(name="xstage", bufs=B_LOCAL) as xstage,
            tc.tile_pool(name="obuf", bufs=5) as opool,
            tc.tile_pool(name="psum", bufs=6, space="PSUM") as pspool,
            tc.tile_pool(name="psum_tr", bufs=2, space="PSUM") as trpool,
        ):
            ident = const.tile([128, 128], F32)
            make_identity(nc, ident)
            # Keep the PE busy from the ordering barrier until the first
            # weight/x chunks land (~2us) - also starts the HAM clock ramp
            # (1.2->2.4 GHz after ~3.4us sustained activity) early.
            for _ in range(8):
                warm = trpool.tile([128, 128], F32, tag="tr")
                nc.tensor.transpose(warm, ident, ident)

            w_t = const.tile([128, 18, 128], F16)
            bias_sb = const.tile([128, 2], F32)
            stages = [
                xstage.tile([128, H, WP], F16, name=f"st{b}", tag=f"st{b}")
                for b in range(B_LOCAL)
            ]

            # All loads issued up front, in dependency-priority order.
            # sync ring: first w taps + sample-0 chunks + sample 1.
            nc.sync.dma_start(w_t[:, 0:3], w_d[:, 0:3])
            nc.sync.dma_start(stages[0][:, 0:9], x_d[0, :, 0:9])
            nc.sync.dma_start(w_t[:, 3:9], w_d[:, 3:9])
            for r0, nr in S0_CHUNKS[1:]:
                nc.sync.dma_start(
                    stages[0][:, r0 : r0 + nr], x_d[0, :, r0 : r0 + nr]
                )
            nc.sync.dma_start(stages[1], x_d[1])
            # scalar (ACT) ring: bias + cb1 weights + samples 2-3.
            nc.scalar.dma_start(bias_sb, b_d.rearrange("(cb cp) -> cp cb", cb=2))
            nc.scalar.dma_start(w_t[:, 9:18], w_d[:, 9:18])
            nc.scalar.dma_start(stages[2], x_d[2])
            nc.scalar.dma_start(stages[3], x_d[3])

            o_v = o_d.rearrange("b (cb cp) h w -> b cb cp (h w)", cb=2)

            def kh_order(cb, t):
                # First tap of each group must cover the full PSUM range
                # (start=True clears the whole bank's has_written). kh1 is
                # always full for cb0 (incl. t=0/t=7); kh0 is full for t>0.
                if cb == 0 or t == 0:
                    return (1, 2, 0)
                return (0, 1, 2)

            def emit_taps(ps, st, cb, h0, n_out, kh_seq):
                """n_out output rows starting at h0, into ps[:, :n_out*W]."""
                i = 0
                n_mm = 3 * len(kh_seq)
                for kh in kh_seq:
                    r0 = h0 + kh - 1
                    rs, re = max(r0, 0), min(r0 + n_out, H)
                    a = (rs - r0) * W
                    b_ = a + (re - rs) * W
                    for kw in range(3):
                        nc.tensor.matmul(
                            ps[:, a:b_],
                            w_t[:, TAP[(cb, kh)] + kw, :],
                            st[:, rs:re, kw : kw + W],
                            start=(i == 0),
                            stop=(i == n_mm - 1),
                        )
                        i += 1

            # Output tiles are drained (bias-add, fp16 cast) per PSUM tile
            # but stored one pair (t even, t odd) at a time: half the DMA
            # issues and completion semaphores.
            pair_obs = {}

            def conv_tile(b, cb, t):
                st = stages[b]
                h0 = t * ROWS
                ps = pspool.tile([128, ROWS * W], F32)
                emit_taps(ps, st, cb, h0, ROWS, kh_order(cb, t))
                key = (b, cb, t // 2)
                if key not in pair_obs:
                    pair_obs[key] = opool.tile(
                        [128, 2 * ROWS * W], F16, name="ob", tag="ob"
                    )
                ob = pair_obs[key]
                half = t % 2
                sl = slice(half * ROWS * W, (half + 1) * ROWS * W)
                nc.scalar.add(ob[:, sl], ps, bias_sb[:, cb : cb + 1])
                if half == 1:
                    # Alternate pair stores across both HWDGE rings: halves
                    # per-ring serialization and overlaps the final stores.
                    eng = nc.scalar if (b + cb + t // 2) % 2 else nc.sync
                    eng.dma_start(
                        o_v[b, cb, :, (t - 1) * ROWS * W : (t + 1) * ROWS * W], ob
                    )

            def penultimate_tile(b, cb, t):
                # Pair partner of the final tile: store alone so the final
                # tile can stream out in small strips.
                st = stages[b]
                h0 = t * ROWS
                ps = pspool.tile([128, ROWS * W], F32, name="ps")
                emit_taps(ps, st, cb, h0, ROWS, kh_order(cb, t))
                ob = opool.tile([128, ROWS * W], F16, name="obp", tag="obt")
                nc.scalar.add(ob, ps, bias_sb[:, cb : cb + 1])
                nc.sync.dma_start(o_v[b, cb, :, h0 * W : (h0 + ROWS) * W], ob)

            def final_tile(b, cb, t):
                # 4+2+2 rows: each strip's drain+store+completion hides
                # under the next strip's matmuls, shortening the
                # end-of-kernel chain.
                st = stages[b]
                h0 = t * ROWS
                strips = [(h0, 4), (h0 + 4, 2), (h0 + 6, 2)]
                for si, (hh0, nr) in enumerate(strips):
                    ps = pspool.tile([128, ROWS * W], F32, name="ps")
                    emit_taps(ps[:, : nr * W], st, cb, hh0, nr, (0, 1, 2))
                    ob = opool.tile([128, nr * W], F16, name="obq", tag="obt")
                    o_ap = o_v[b, cb, :, hh0 * W : (hh0 + nr) * W]
                    if si == 2:
                        nc.vector.tensor_scalar_add(
                            ob, ps[:, : nr * W], bias_sb[:, cb : cb + 1]
                        )
                        nc.sync.dma_start(o_ap, ob)
                    else:
                        nc.scalar.add(ob, ps[:, : nr * W], bias_sb[:, cb : cb + 1])
                        (nc.sync if si == 0 else nc.scalar).dma_start(o_ap, ob)

            n_total = 2 * N_T * B_LOCAL
            n_done = 0
            for b in range(B_LOCAL):
                for cb in range(2):
                    for t in range(N_T):
                        if n_done == n_total - 2:
                            penultimate_tile(b, cb, t)
                        elif n_done == n_total - 1:
                            final_tile(b, cb, t)
                        else:
                            conv_tile(b, cb, t)
                        n_done += 1

    nc.finalize()
    return nc


def run(x: np.ndarray, weight: np.ndarray, bias: np.ndarray, **spmd_kwargs):
    weight = np.ascontiguousarray(weight, dtype=np.float32)
    bias = np.ascontiguousarray(bias, dtype=np.float32)

    # Host-side x pad: [B, CI, H, W] f32 -> [B, CI, H, W+2] f16, zero edge cols.
    x_pad = np.zeros((B_FULL, CI, H, WP), dtype=np.float16)
    x_pad[:, :, :, 1 : W + 1] = x

    # Host-side weight repack: [co, ci, kh, kw] -> [ci, tap, cp] fp16 with
    # tap order [cb0 kh1 | cb0 kh2 | cb0 kh0 | cb1 kh0..kh2], kw-minor.
    w5 = weight.reshape(2, CO // 2, CI, 3, 3).transpose(0, 3, 4, 2, 1)
    # w5: [cb, kh, kw, ci, cp]
    wt = np.concatenate(
        [w5[0][[1, 2, 0]].reshape(9, CI, CO // 2), w5[1].reshape(9, CI, CO // 2)],
        axis=0,
    )
    wt = np.ascontiguousarray(wt.transpose(1, 0, 2)).astype(np.float16)

    nc = build_nc()
    in_maps = [
        {
            "x": x_pad[c * B_LOCAL : (c + 1) * B_LOCAL],
            "wt": wt,
            "bias": bias,
        }
        for c in range(N_CORES)
    ]
    res = run_bass_kernel_spmd(
        nc, in_maps, core_ids=list(range(N_CORES)), **spmd_kwargs
    )
    out = np.concatenate(
        [np.asarray(r["out"]).astype(np.float32) for r in res.results], axis=0
    )
    return out, res


def kernel(x: np.ndarray, weight: np.ndarray, bias: np.ndarray) -> np.ndarray:
    out, _ = run(x, weight, bias)
    return out


# revision 6
# speedup vs baseline: 1.0021x; 1.0021x over previous
"""Round 4: host-padded x (W+2), strided matmul views, no on-chip copies.

Trainium2 Bass kernel for a 3x3 stride-1 pad-1 conv:
x (32,128,64,64) f32, weight (256,128,3,3) f32, bias (256,) f32
-> out (32,256,64,64) f32.

Data-parallel over batch across 8 NeuronCores (4 samples each); conv as
9 shifted fp16 matmuls per 8-row output tile accumulating in fp32 PSUM.

Horizontal padding comes for free: the host pads x to width 66 with zero
columns, so tap (kh, kw) is just the strided SBUF view
st[:, r0:r0+8, kw:kw+64] (row stride 66) - no shifted copies, no memsets.
Vertical padding via range-restricted matmuls on edge tiles (taps ordered
so the first matmul of each accumulation group covers the full PSUM range).

Weights are repacked on the host to wt[ci, tap, co_half] fp16 with the
first-needed taps (cb0 kh1) first; the first weight chunk and the first
x rows go out first on the sync DGE ring so conv matmuls can start as
soon as the ring delivers (~2.3us after issue). A few identity transposes
keep the PE busy from the ordering barrier until then, which also starts
the HAM clock ramp early.

Output pairs (two 8-row tiles) are bias-added on the scalar engine and
stored fp16, alternating between both DGE rings; the final tile is split
4+2+2 rows so the closing matmul->drain->store->completion chain is short.
"""

import numpy as np

import concourse.bass as bass
from concourse import bacc
import concourse.mybir as mybir
import concourse.tile as tile
from concourse.bass_utils import run_bass_kernel_spmd
from concourse.masks import make_identity

N_CORES = 8
B_FULL = 32
B_LOCAL = B_FULL // N_CORES  # 4
CI = 128
CO = 256
H = W = 64
WP = W + 2  # host-padded width (zero col at 0 and 65)
ROWS = 8  # output rows per PSUM tile -> free dim 8*64 = 512
N_T = H // ROWS  # 8
F32 = mybir.dt.float32
F16 = mybir.dt.float16

# wt tap layout: [cb0 kh1 | cb0 kh2 | cb0 kh0 | cb1 kh0 | cb1 kh1 | cb1 kh2]
# (first-needed taps first; each group is kw=0,1,2)
TAP = {(0, 1): 0, (0, 2): 3, (0, 0): 6, (1, 0): 9, (1, 1): 12, (1, 2): 15}

# Sample-0 x arrives in four row chunks so compute can start early.
# Tile t needs input rows 8t-1 .. 8t+8.
S0_CHUNKS = [(0, 9), (9, 8), (17, 16), (33, 31)]


def build_nc():
    nc = bacc.Bacc()
    x_d = nc.dram_tensor("x", [B_LOCAL, CI, H, WP], F16, kind="ExternalInput")
    w_d = nc.dram_tensor("wt", [CI, 18, CO // 2], F16, kind="ExternalInput")
    b_d = nc.dram_tensor("bias", [CO], F32, kind="ExternalInput")
    o_d = nc.dram_tensor("out", [B_LOCAL, CO, H, W], F16, kind="ExternalOutput")

    with tile.TileContext(nc) as tc:
        with (
            tc.tile_pool(name="const", bufs=1) as const,
            tc.tile_pool(name="xstage", bufs=B_LOCAL) as xstage,
            tc.tile_pool(name="obuf", bufs=5) as opool,
            tc.tile_pool(name="psum", bufs=6, space="PSUM") as pspool,
            tc.tile_pool(name="psum_tr", bufs=2, space="PSUM") as trpool,
        ):
            ident = const.tile([128, 128], F32)
            make_identity(nc, ident)
            # Keep the PE busy from the ordering barrier until the first
            # weight/x chunks land (~2.4us) - also starts the HAM clock ramp
            # (1.2->2.4 GHz after ~3.4us sustained activity) early. A gap
            # here resets the ramp window, so slightly over-provision.
            for _ in range(11):
                warm = trpool.tile([128, 128], F32, tag="tr")
                nc.tensor.transpose(warm, ident, ident)

            w_t = const.tile([128, 18, 128], F16)
            bias_sb = const.tile([128, 2], F32)
            stages = [
                xstage.tile([128, H, WP], F16, name=f"st{b}", tag=f"st{b}")
                for b in range(B_LOCAL)
            ]

            # Critical loads first. Both DGE rings share the same 16 DMA
            # engines, so the bulk sample loads (stages 1-3) are deferred
            # into the store stream below - issuing them here would starve
            # the first w/x chunks and stall the PE mid-clock-ramp.
            # sync ring: first w taps + sample-0 row chunks.
            nc.sync.dma_start(w_t[:, 0:3], w_d[:, 0:3])
            nc.sync.dma_start(stages[0][:, 0:9], x_d[0, :, 0:9])
            nc.sync.dma_start(w_t[:, 3:9], w_d[:, 3:9])
            for r0, nr in S0_CHUNKS[1:]:
                nc.sync.dma_start(
                    stages[0][:, r0 : r0 + nr], x_d[0, :, r0 : r0 + nr]
                )
            # scalar (ACT) ring: bias + cb1 weights.
            nc.scalar.dma_start(bias_sb, b_d.rearrange("(cb cp) -> cp cb", cb=2))
            nc.scalar.dma_start(w_t[:, 9:18], w_d[:, 9:18])

            o_v = o_d.rearrange("b (cb cp) h w -> b cb cp (h w)", cb=2)

            def kh_order(cb, t):
                # First tap of each group must cover the full PSUM range
                # (start=True clears the whole bank's has_written). kh1 is
                # always full for cb0 (incl. t=0/t=7); kh0 is full for t>0.
                if cb == 0 or t == 0:
                    return (1, 2, 0)
                return (0, 1, 2)

            def emit_taps(ps, st, cb, h0, n_out, kh_seq):
                """n_out output rows starting at h0, into ps[:, :n_out*W]."""
                i = 0
                n_mm = 3 * len(kh_seq)
                for kh in kh_seq:
                    r0 = h0 + kh - 1
                    rs, re = max(r0, 0), min(r0 + n_out, H)
                    a = (rs - r0) * W
                    b_ = a + (re - rs) * W
                    for kw in range(3):
                        nc.tensor.matmul(
                            ps[:, a:b_],
                            w_t[:, TAP[(cb, kh)] + kw, :],
                            st[:, rs:re, kw : kw + W],
                            start=(i == 0),
                            stop=(i == n_mm - 1),
                        )
                        i += 1

            # Output tiles are drained (bias-add, fp16 cast) per PSUM tile
            # but stored one pair (t even, t odd) at a time: half the DMA
            # issues and completion semaphores.
            pair_obs = {}
            store_count = {"sync": 0, "scalar": 0}
            # Deferred bulk loads: (ring, nth store on that ring) -> sample.
            deferred_loads = {("sync", 1): 1, ("scalar", 1): 2, ("scalar", 2): 3}

            def conv_tile(b, cb, t):
                st = stages[b]
                h0 = t * ROWS
                ps = pspool.tile([128, ROWS * W], F32)
                emit_taps(ps, st, cb, h0, ROWS, kh_order(cb, t))
                key = (b, cb, t // 2)
                if key not in pair_obs:
                    pair_obs[key] = opool.tile(
                        [128, 2 * ROWS * W], F16, name="ob", tag="ob"
                    )
                ob = pair_obs[key]
                half = t % 2
                sl = slice(half * ROWS * W, (half + 1) * ROWS * W)
                nc.scalar.add(ob[:, sl], ps, bias_sb[:, cb : cb + 1])
                if half == 1:
                    # Alternate pair stores across both HWDGE rings: halves
                    # per-ring serialization and overlaps the final stores.
                    ring = "scalar" if (b + cb + t // 2) % 2 else "sync"
                    eng = getattr(nc, ring)
                    eng.dma_start(
                        o_v[b, cb, :, (t - 1) * ROWS * W : (t + 1) * ROWS * W], ob
                    )
                    store_count[ring] += 1
                    s = deferred_loads.pop((ring, store_count[ring]), None)
                    if s is not None:
                        eng.dma_start(stages[s], x_d[s])

            def penultimate_tile(b, cb, t):
                # Pair partner of the final tile: store alone so the final
                # tile can stream out in small strips.
                st = stages[b]
                h0 = t * ROWS
                ps = pspool.tile([128, ROWS * W], F32, name="ps")
                emit_taps(ps, st, cb, h0, ROWS, kh_order(cb, t))
                ob = opool.tile([128, ROWS * W], F16, name="obp", tag="obt")
                nc.scalar.add(ob, ps, bias_sb[:, cb : cb + 1])
                nc.sync.dma_start(o_v[b, cb, :, h0 * W : (h0 + ROWS) * W], ob)

            def final_tile(b, cb, t):
                # 4+2+2 rows: each strip's drain+store+completion hides
                # under the next strip's matmuls, shortening the
                # end-of-kernel chain.
                st = stages[b]
                h0 = t * ROWS
                strips = [(h0, 4), (h0 + 4, 2), (h0 + 6, 2)]
                for si, (hh0, nr) in enumerate(strips):
                    ps = pspool.tile([128, ROWS * W], F32, name="ps")
                    emit_taps(ps[:, : nr * W], st, cb, hh0, nr, (0, 1, 2))
                    ob = opool.tile([128, nr * W], F16, name="obq", tag="obt")
                    o_ap = o_v[b, cb, :, hh0 * W : (hh0 + nr) * W]
                    if si == 2:
                        nc.vector.tensor_scalar_add(
                            ob, ps[:, : nr * W], bias_sb[:, cb : cb + 1]
                        )
                        nc.sync.dma_start(o_ap, ob)
                    else:
                        nc.scalar.add(ob, ps[:, : nr * W], bias_sb[:, cb : cb + 1])
                        (nc.sync if si == 0 else nc.scalar).dma_start(o_ap, ob)

            n_total = 2 * N_T * B_LOCAL
            n_done = 0
            for b in range(B_LOCAL):
                for cb in range(2):
                    for t in range(N_T):
                        if n_done == n_total - 2:
                            penultimate_tile(b, cb, t)
                        elif n_done == n_total - 1:
                            final_tile(b, cb, t)
                        else:
                            conv_tile(b, cb, t)
                        n_done += 1

    nc.finalize()
    return nc


def run(x: np.ndarray, weight: np.ndarray, bias: np.ndarray, **spmd_kwargs):
    weight = np.ascontiguousarray(weight, dtype=np.float32)
    bias = np.ascontiguousarray(bias, dtype=np.float32)

    # Host-side x pad: [B, CI, H, W] f32 -> [B, CI, H, W+2] f16, zero edge cols.
    x_pad = np.zeros((B_FULL, CI, H, WP), dtype=np.float16)
    x_pad[:, :, :, 1 : W + 1] = x

    # Host-side weight repack: [co, ci, kh, kw] -> [ci, tap, cp] fp16 with
    # tap order [cb0 kh1 | cb0 kh2 | cb0 kh0 | cb1 kh0..kh2], kw-minor.
    w5 = weight.reshape(2, CO // 2, CI, 3, 3).transpose(0, 3, 4, 2, 1)
    # w5: [cb, kh, kw, ci, cp]
    wt = np.concatenate(
        [w5[0][[1, 2, 0]].reshape(9, CI, CO // 2), w5[1].reshape(9, CI, CO // 2)],
        axis=0,
    )
    wt = np.ascontiguousarray(wt.transpose(1, 0, 2)).astype(np.float16)

    nc = build_nc()
    in_maps = [
        {
            "x": x_pad[c * B_LOCAL : (c + 1) * B_LOCAL],
            "wt": wt,
            "bias": bias,
        }
        for c in range(N_CORES)
    ]
    res = run_bass_kernel_spmd(
        nc, in_maps, core_ids=list(range(N_CORES)), **spmd_kwargs
    )
    out = np.concatenate(
        [np.asarray(r["out"]).astype(np.float32) for r in res.results], axis=0
    )
    return out, res


def kernel(x: np.ndarray, weight: np.ndarray, bias: np.ndarray) -> np.ndarray:
    out, _ = run(x, weight, bias)
    return out


# revision 10
# speedup vs baseline: 1.0238x; 1.0217x over previous
"""Round 4: host-padded x (W+2), strided matmul views, no on-chip copies.

Trainium2 Bass kernel for a 3x3 stride-1 pad-1 conv:
x (32,128,64,64) f32, weight (256,128,3,3) f32, bias (256,) f32
-> out (32,256,64,64) f32.

Data-parallel over batch across 8 NeuronCores (4 samples each); conv as
9 shifted fp16 matmuls per 8-row output tile accumulating in fp32 PSUM.

Horizontal padding comes for free: the host pads x to width 66 with zero
columns, so tap (kh, kw) is just the strided SBUF view
st[:, r0:r0+8, kw:kw+64] (row stride 66) - no shifted copies, no memsets.
Vertical padding via range-restricted matmuls on edge tiles (taps ordered
so the first matmul of each accumulation group covers the full PSUM range).

Weights are repacked on the host to wt[ci, tap, co_half] fp16 with the
first-needed taps (cb0 kh1) first; the first weight chunk and the first
x rows go out first on the sync DGE ring so conv matmuls can start as
soon as the ring delivers (~2.3us after issue). A few identity transposes
keep the PE busy from the ordering barrier until then, which also starts
the HAM clock ramp early.

Output pairs (two 8-row tiles) are bias-added on the scalar engine and
stored fp16, alternating between both DGE rings; the final tile is split
4+2+2 rows so the closing matmul->drain->store->completion chain is short.
"""

import numpy as np

import concourse.bass as bass
from concourse import bacc
import concourse.mybir as mybir
import concourse.tile as tile
from concourse.bass_utils import run_bass_kernel_spmd
from concourse.masks import make_identity

N_CORES = 8
B_FULL = 32
B_LOCAL = B_FULL // N_CORES  # 4
CI = 128
CO = 256
H = W = 64
WP = W + 2  # host-padded width (zero col at 0 and 65)
ROWS = 8  # output rows per PSUM tile -> free dim 8*64 = 512
N_T = H // ROWS  # 8
F32 = mybir.dt.float32
F16 = mybir.dt.float16

# wt tap layout: [cb0 kh1 | cb0 kh2 | cb0 kh0 | cb1 kh0 | cb1 kh1 | cb1 kh2]
# (first-needed taps first; each group is kw=0,1,2)
TAP = {(0, 1): 0, (0, 2): 3, (0, 0): 6, (1, 0): 9, (1, 1): 12, (1, 2): 15}

# Sample-0 x arrives in four row chunks so compute can start early.
# Tile t needs input rows 8t-1 .. 8t+8.
S0_CHUNKS = [(0, 9), (9, 8), (17, 16), (33, 31)]


def build_nc():
    nc = bacc.Bacc()
    x_d = nc.dram_tensor("x", [B_LOCAL, CI, H, WP], F16, kind="ExternalInput")
    w_d = nc.dram_tensor("wt", [CI, 18, CO // 2], F16, kind="ExternalInput")
    b_d = nc.dram_tensor("bias", [CO], F32, kind="ExternalInput")
    o_d = nc.dram_tensor("out", [B_LOCAL, CO, H, W], F16, kind="ExternalOutput")

    with tile.TileContext(nc) as tc:
        with (
            tc.tile_pool(name="const", bufs=1) as const,
            tc.tile_pool(name="xstage", bufs=B_LOCAL) as xstage,
            tc.tile_pool(name="obuf", bufs=5) as opool,
            tc.tile_pool(name="psum", bufs=6, space="PSUM") as pspool,
            tc.tile_pool(name="psum_tr", bufs=2, space="PSUM") as trpool,
        ):
            ident = const.tile([128, 128], F32)
            make_identity(nc, ident)
            # Keep the PE busy from the ordering barrier until the first
            # weight/x chunks land (~2.4us) - also starts the HAM clock ramp
            # (1.2->2.4 GHz after ~3.4us sustained activity) early. A gap
            # here resets the ramp window, so slightly over-provision.
            for _ in range(10):
                warm = trpool.tile([128, 128], F32, tag="tr")
                nc.tensor.transpose(warm, ident, ident)

            w_t = const.tile([128, 18, 128], F16)
            bias_sb = const.tile([128, 2], F32)
            stages = [
                xstage.tile([128, H, WP], F16, name=f"st{b}", tag=f"st{b}")
                for b in range(B_LOCAL)
            ]

            # Critical loads first. Both DGE rings share the same 16 DMA
            # engines, so the bulk sample loads (stages 1-3) are deferred
            # into the store stream below - issuing them here would starve
            # the first w/x chunks and stall the PE mid-clock-ramp.
            # sync ring: first w taps + sample-0 row chunks.
            nc.sync.dma_start(w_t[:, 0:3], w_d[:, 0:3])
            nc.sync.dma_start(stages[0][:, 0:9], x_d[0, :, 0:9])
            nc.sync.dma_start(w_t[:, 3:9], w_d[:, 3:9])
            for r0, nr in S0_CHUNKS[1:]:
                nc.sync.dma_start(
                    stages[0][:, r0 : r0 + nr], x_d[0, :, r0 : r0 + nr]
                )
            # scalar (ACT) ring: bias + cb1 weights.
            nc.scalar.dma_start(bias_sb, b_d.rearrange("(cb cp) -> cp cb", cb=2))
            nc.scalar.dma_start(w_t[:, 9:18], w_d[:, 9:18])
            # Bulk sample loads, placed behind the first output stores via
            # scheduling-time hints (emission order alone gets hoisted by
            # the tile scheduler). At runtime each lands behind a store
            # whose drain fires at ~15-25us - long before samples 1-3 are
            # consumed (~42/73/103us) - keeping the early DMA rings clear
            # for the critical w/x0 chunks.
            with tc.tile_wait_until(0.016):
                nc.sync.dma_start(stages[1], x_d[1])
            with tc.tile_wait_until(0.018):
                nc.scalar.dma_start(stages[2], x_d[2])
            with tc.tile_wait_until(0.025):
                nc.scalar.dma_start(stages[3], x_d[3])

            o_v = o_d.rearrange("b (cb cp) h w -> b cb cp (h w)", cb=2)

            def kh_order(cb, t):
                # First tap of each group must cover the full PSUM range
                # (start=True clears the whole bank's has_written). kh1 is
                # always full for cb0 (incl. t=0/t=7); kh0 is full for t>0.
                if cb == 0 or t == 0:
                    return (1, 2, 0)
                return (0, 1, 2)

            def emit_taps(ps, st, cb, h0, n_out, kh_seq):
                """n_out output rows starting at h0, into ps[:, :n_out*W]."""
                i = 0
                n_mm = 3 * len(kh_seq)
                for kh in kh_seq:
                    r0 = h0 + kh - 1
                    rs, re = max(r0, 0), min(r0 + n_out, H)
                    a = (rs - r0) * W
                    b_ = a + (re - rs) * W
                    for kw in range(3):
                        nc.tensor.matmul(
                            ps[:, a:b_],
                            w_t[:, TAP[(cb, kh)] + kw, :],
                            st[:, rs:re, kw : kw + W],
                            start=(i == 0),
                            stop=(i == n_mm - 1),
                        )
                        i += 1

            # Output tiles are drained (bias-add, fp16 cast) per PSUM tile
            # but stored one pair (t even, t odd) at a time: half the DMA
            # issues and completion semaphores.
            pair_obs = {}

            def conv_tile(b, cb, t):
                st = stages[b]
                h0 = t * ROWS
                ps = pspool.tile([128, ROWS * W], F32)
                emit_taps(ps, st, cb, h0, ROWS, kh_order(cb, t))
                key = (b, cb, t // 2)
                if key not in pair_obs:
                    pair_obs[key] = opool.tile(
                        [128, 2 * ROWS * W], F16, name="ob", tag="ob"
                    )
                ob = pair_obs[key]
                half = t % 2
                sl = slice(half * ROWS * W, (half + 1) * ROWS * W)
                nc.scalar.add(ob[:, sl], ps, bias_sb[:, cb : cb + 1])
                if half == 1:
                    # Alternate pair stores across both HWDGE rings: halves
                    # per-ring serialization and overlaps the final stores.
                    eng = nc.scalar if (b + cb + t // 2) % 2 else nc.sync
                    eng.dma_start(
                        o_v[b, cb, :, (t - 1) * ROWS * W : (t + 1) * ROWS * W], ob
                    )

            def penultimate_tile(b, cb, t):
                # Pair partner of the final tile: store alone so the final
                # tile can stream out in small strips.
                st = stages[b]
                h0 = t * ROWS
                ps = pspool.tile([128, ROWS * W], F32, name="ps")
                emit_taps(ps, st, cb, h0, ROWS, kh_order(cb, t))
                ob = opool.tile([128, ROWS * W], F16, name="obp", tag="obt")
                nc.scalar.add(ob, ps, bias_sb[:, cb : cb + 1])
                nc.sync.dma_start(o_v[b, cb, :, h0 * W : (h0 + ROWS) * W], ob)

            def final_tile(b, cb, t):
                # 4+2+2 rows: each strip's drain+store+completion hides
                # under the next strip's matmuls, shortening the
                # end-of-kernel chain.
                st = stages[b]
                h0 = t * ROWS
                strips = [(h0, 4), (h0 + 4, 2), (h0 + 6, 2)]
                for si, (hh0, nr) in enumerate(strips):
                    ps = pspool.tile([128, ROWS * W], F32, name="ps")
                    emit_taps(ps[:, : nr * W], st, cb, hh0, nr, (0, 1, 2))
                    ob = opool.tile([128, nr * W], F16, name="obq", tag="obt")
                    o_ap = o_v[b, cb, :, hh0 * W : (hh0 + nr) * W]
                    if si == 2:
                        nc.vector.tensor_scalar_add(
                            ob, ps[:, : nr * W], bias_sb[:, cb : cb + 1]
                        )
                        nc.sync.dma_start(o_ap, ob)
                    else:
                        nc.scalar.add(ob, ps[:, : nr * W], bias_sb[:, cb : cb + 1])
                        (nc.sync if si == 0 else nc.scalar).dma_start(o_ap, ob)

            n_total = 2 * N_T * B_LOCAL
            n_done = 0
            for b in range(B_LOCAL):
                for cb in range(2):
                    for t in range(N_T):
                        if n_done == n_total - 2:
                            penultimate_tile(b, cb, t)
                        elif n_done == n_total - 1:
                            final_tile(b, cb, t)
                        else:
                            conv_tile(b, cb, t)
                        n_done += 1

    nc.finalize()
    return nc


def run(x: np.ndarray, weight: np.ndarray, bias: np.ndarray, **spmd_kwargs):
    weight = np.ascontiguousarray(weight, dtype=np.float32)
    bias = np.ascontiguousarray(bias, dtype=np.float32)

    # Host-side x pad: [B, CI, H, W] f32 -> [B, CI, H, W+2] f16, zero edge cols.
    x_pad = np.zeros((B_FULL, CI, H, WP), dtype=np.float16)
    x_pad[:, :, :, 1 : W + 1] = x

    # Host-side weight repack: [co, ci, kh, kw] -> [ci, tap, cp] fp16 with
    # tap order [cb0 kh1 | cb0 kh2 | cb0 kh0 | cb1 kh0..kh2], kw-minor.
    w5 = weight.reshape(2, CO // 2, CI, 3, 3).transpose(0, 3, 4, 2, 1)
    # w5: [cb, kh, kw, ci, cp]
    wt = np.concatenate(
        [w5[0][[1, 2, 0]].reshape(9, CI, CO // 2), w5[1].reshape(9, CI, CO // 2)],
        axis=0,
    )
    wt = np.ascontiguousarray(wt.transpose(1, 0, 2)).astype(np.float16)

    nc = build_nc()
    in_maps = [
        {
            "x": x_pad[c * B_LOCAL : (c + 1) * B_LOCAL],
            "wt": wt,
            "bias": bias,
        }
        for c in range(N_CORES)
    ]
    res = run_bass_kernel_spmd(
        nc, in_maps, core_ids=list(range(N_CORES)), **spmd_kwargs
    )
    out = np.concatenate(
        [np.asarray(r["out"]).astype(np.float32) for r in res.results], axis=0
    )
    return out, res


def kernel(x: np.ndarray, weight: np.ndarray, bias: np.ndarray) -> np.ndarray:
    out, _ = run(x, weight, bias)
    return out
